# revision 5
# baseline (speedup 1.0000x reference)
"""Trainium2 Bass kernel for nn_CAMD_32349693673977 (sparse cross-modal linear attention).

Self-contained: accepts FULL inputs (as produced by the problem's setup_inputs()),
shards across 8 NeuronCores internally (sequence-parallel over time with an
AllGather'd KV-state carry), runs one SPMD Bass/Tile kernel, gathers the output.

Math: out[i] = Q_i @ sum_m sum_{j: t2_m[j] <= t1[i]} K_m[j] V_m[j]^T
  Q = elu(mlp(X_ref))+1, K_m = elu(mlp(X_m))+1, V_m = mlp(X_m)

Device algorithm per core (rows [1024c, 1024(c+1))):
  - mod events are time-split at ref-row boundaries (host searchsorted), padded
    to a fixed per-core capacity with t=+1e9 sentinel events
  - 2-layer MLPs on PE (K/V layer-1 packed into one 128-wide matmul)
  - per-128-chunk KV outer-product sums -> local prefix sums (DVE)
  - AllGather of per-core KV totals -> masked prefix sum = cross-core carry
  - per 128-row ref chunk: one state matmul Q_I @ S_prefix plus a few
    time-masked boundary attention pairs (K_J^T Q_I) .* mask -> @ V_J
"""

import os
import sys

import numpy as np

if "/opt/trn_rl_repo" not in sys.path:
    sys.path.insert(0, "/opt/trn_rl_repo")

import concourse.bacc as bacc
import concourse.bass as bass
import concourse.tile as tile
from concourse import mybir
from concourse.bass_utils import run_bass_kernel_spmd

F32 = mybir.dt.float32
ALU = mybir.AluOpType
ACT = mybir.ActivationFunctionType

N_CORES = 8
R = 1024          # ref rows per core
RC = 8            # 128-row ref chunks per core
D = 64            # d_qk == d_v
PAD_T = 1.0e9

# modality order and per-core padded capacities (in 128-event chunks)
MODS = [("reference", 64), ("m1", 32), ("m2", 48)]
BASE_NCH = {"reference": 9, "m1": 10, "m2": 8}


def _mlp_layers(layers):
    (w1, b1), (w2, b2) = layers
    return (np.asarray(w1, np.float32), np.asarray(b1, np.float32),
            np.asarray(w2, np.float32), np.asarray(b2, np.float32))


def _slices(n, step=512):
    out = []
    i = 0
    while i < n:
        out.append((i, min(i + step, n)))
        i += step
    return out


def build_program(nch, bands):
    """Build the SPMD Bass program.

    nch:   {mod: chunk count} local capacity per core
    bands: {mod: (j0[RC], j1[RC])} shared static band table
    """
    nc = bacc.Bacc("TRN2", target_bir_lowering=False, debug=False,
                   num_devices=N_CORES)

    # ---- I/O declarations ----
    inp = {}

    def dram_in(name, shape):
        inp[name] = nc.dram_tensor(name, list(shape), F32, kind="ExternalInput")
        return inp[name]

    dram_in("xq_t", (D, R))
    for m, d in MODS:
        cap = nch[m] * 128
        dram_in(f"x_{m}_t", (d, cap))
        dram_in(f"t2n_{m}", (128, nch[m]))
        dram_in(f"rmask_{m}", (128, nch[m]))
        dram_in(f"w1kv_{m}", (d, 128))      # [w1k | w1v] packed
        dram_in(f"b1kv_{m}", (128, 1))
        dram_in(f"w2k_{m}", (D, D))
        dram_in(f"b2k_{m}", (D, 1))
        dram_in(f"w2v_{m}", (D, D))
        dram_in(f"b2v_{m}", (D, 1))         # used via broadcast DMA
    dram_in("w1q", (D, D))
    dram_in("b1q", (D, 1))
    dram_in("w2q", (D, D))
    dram_in("b2q", (D, 1))
    dram_in("prefmask", (D, N_CORES))
    dram_in("ident", (D, D))

    out_t = nc.dram_tensor("o", [R, D], F32, kind="ExternalOutput")

    with tile.TileContext(nc) as tc:
        with (
            tc.tile_pool(name="const", bufs=1) as const,
            tc.tile_pool(name="work", bufs=3) as work,
            tc.tile_pool(name="ps_wide", bufs=2, space="PSUM") as ps_wide,
            tc.tile_pool(name="ps_at", bufs=3, space="PSUM") as ps_at,
            tc.tile_pool(name="ps_o", bufs=2, space="PSUM") as ps_o,
            tc.tile_pool(name="dram", bufs=1, space="DRAM") as dram,
        ):
            # ---- load constants / inputs into SBUF ----
            def load(name, shape):
                t = const.tile(list(shape), F32, tag=name)
                nc.sync.dma_start(out=t[:], in_=inp[name][:])
                return t

            xq_t = load("xq_t", (D, R))
            w1q = load("w1q", (D, D))
            b1q = load("b1q", (D, 1))
            w2q = load("w2q", (D, D))
            b2q = load("b2q", (D, 1))
            prefmask = load("prefmask", (D, N_CORES))
            ident = load("ident", (D, D))

            mt = {}
            for m, d in MODS:
                cap = nch[m] * 128
                mt[m] = dict(
                    x_t=load(f"x_{m}_t", (d, cap)),
                    t2n=load(f"t2n_{m}", (128, nch[m])),
                    rmask=load(f"rmask_{m}", (128, nch[m])),
                    w1kv=load(f"w1kv_{m}", (d, 128)),
                    b1kv=load(f"b1kv_{m}", (128, 1)),
                    w2k=load(f"w2k_{m}", (D, D)),
                    b2k=load(f"b2k_{m}", (D, 1)),
                )
                # w2v lives in partitions 64:128 so it base-matches hkv[64:128]
                w2v_hi = const.tile([128, D], F32, tag=f"w2v_{m}")
                nc.sync.dma_start(out=w2v_hi[64:128, :], in_=inp[f"w2v_{m}"][:])
                mt[m]["w2v"] = w2v_hi
                # broadcast b2v across 128 partitions: (128, 64)
                h = inp[f"b2v_{m}"]
                bcast = bass.AP(tensor=h.ap().tensor, offset=0,
                                ap=[[0, 128], [1, D]])
                t = const.tile([128, D], F32, tag=f"b2v_bc_{m}")
                nc.sync.dma_start(out=t[:], in_=bcast)
                mt[m]["b2v_bc"] = t

            # t1 broadcast across partitions: (128, 1024) from xq_t row 63
            t1b = const.tile([128, R], F32, tag="t1b")
            h = inp["xq_t"]
            t1_bcast = bass.AP(tensor=h.ap().tensor, offset=(D - 1) * R,
                               ap=[[0, 128], [1, R]])
            nc.sync.dma_start(out=t1b[:], in_=t1_bcast)

            # ---- per-modality K/V pipeline ----
            for m, d in MODS:
                cap = nch[m] * 128
                v = mt[m]
                # layer 1 (K and V packed): H_kv^T (128, cap)
                hkv = const.tile([128, cap], F32, tag=f"hkv_{m}")
                for s0, s1 in _slices(cap):
                    ps = ps_wide.tile([128, 512], F32, tag="w")
                    nc.tensor.matmul(ps[:, : s1 - s0], v["w1kv"][:],
                                     v["x_t"][:, s0:s1], start=True, stop=True)
                    nc.scalar.activation(out=hkv[:, s0:s1], in_=ps[:, : s1 - s0],
                                         func=ACT.Relu, bias=v["b1kv"][:], scale=1.0)
                # layer 2 K^T (64, cap) with feature map elu+1
                kt = const.tile([D, cap], F32, tag=f"kt_{m}")
                for s0, s1 in _slices(cap):
                    ps = ps_wide.tile([D, 512], F32, tag="w")
                    nc.tensor.matmul(ps[:, : s1 - s0], v["w2k"][:],
                                     hkv[:D, s0:s1], start=True, stop=True)
                    r = work.tile([D, 512], F32, tag="fm_r")
                    nc.scalar.activation(out=r[:, : s1 - s0], in_=ps[:, : s1 - s0],
                                         func=ACT.Relu, bias=v["b2k"][:], scale=1.0)
                    mn = work.tile([D, 512], F32, tag="fm_m")
                    nc.vector.tensor_scalar(out=mn[:, : s1 - s0], in0=ps[:, : s1 - s0],
                                            scalar1=v["b2k"][:], scalar2=0.0,
                                            op0=ALU.add, op1=ALU.min)
                    e = work.tile([D, 512], F32, tag="fm_e")
                    nc.scalar.activation(out=e[:, : s1 - s0], in_=mn[:, : s1 - s0],
                                         func=ACT.Exp)
                    nc.vector.tensor_tensor(out=kt[:, s0:s1], in0=e[:, : s1 - s0],
                                            in1=r[:, : s1 - s0], op=ALU.add)
                # per chunk: V natural, K natural (masked), chunk KV sums
                vn = const.tile([128, D * nch[m]], F32, tag=f"vn_{m}")
                kn = const.tile([128, D * nch[m]], F32, tag=f"kn_{m}")
                ssum = const.tile([D, D * nch[m]], F32, tag=f"ss_{m}")
                for j in range(nch[m]):
                    c0, c1 = j * 128, (j + 1) * 128
                    d0, d1 = j * D, (j + 1) * D
                    psv = ps_at.tile([128, D], F32, tag="at")
                    nc.tensor.matmul(psv[:], hkv[64:128, c0:c1],
                                     v["w2v"][64:128, :], start=True, stop=True)
                    nc.vector.tensor_tensor(out=vn[:, d0:d1], in0=psv[:],
                                            in1=v["b2v_bc"][:], op=ALU.add)
                    pst = ps_at.tile([128, D], F32, tag="at")
                    nc.tensor.transpose(pst[:], kt[:, c0:c1], ident[:])
                    nc.vector.tensor_scalar(out=kn[:, d0:d1], in0=pst[:],
                                            scalar1=v["rmask"][:, j:j + 1],
                                            scalar2=None, op0=ALU.mult)
                    pss = ps_at.tile([D, D], F32, tag="at")
                    nc.tensor.matmul(pss[:], kn[:, d0:d1], vn[:, d0:d1],
                                     start=True, stop=True)
                    nc.scalar.copy(out=ssum[:, d0:d1], in_=pss[:])
                # local prefix sums LS (64, 64*(nch+1)); LS[0] = 0
                ls = const.tile([D, D * (nch[m] + 1)], F32, tag=f"ls_{m}")
                nc.vector.memset(ls[:, 0:D], 0.0)
                for j in range(nch[m]):
                    nc.vector.tensor_tensor(
                        out=ls[:, (j + 1) * D:(j + 2) * D],
                        in0=ls[:, j * D:(j + 1) * D],
                        in1=ssum[:, j * D:(j + 1) * D], op=ALU.add)
                mt[m]["kt"], mt[m]["vn"], mt[m]["ls"] = kt, vn, ls

            # ---- Q pipeline ----
            hq = const.tile([D, R], F32, tag="hq")
            for s0, s1 in _slices(R):
                ps = ps_wide.tile([D, 512], F32, tag="w")
                nc.tensor.matmul(ps[:], w1q[:], xq_t[:, s0:s1], start=True, stop=True)
                nc.scalar.activation(out=hq[:, s0:s1], in_=ps[:], func=ACT.Relu,
                                     bias=b1q[:], scale=1.0)
            qt = const.tile([D, R], F32, tag="qt")
            for s0, s1 in _slices(R):
                ps = ps_wide.tile([D, 512], F32, tag="w")
                nc.tensor.matmul(ps[:], w2q[:], hq[:, s0:s1], start=True, stop=True)
                r = work.tile([D, 512], F32, tag="fm_r")
                nc.scalar.activation(out=r[:], in_=ps[:], func=ACT.Relu,
                                     bias=b2q[:], scale=1.0)
                mn = work.tile([D, 512], F32, tag="fm_m")
                nc.vector.tensor_scalar(out=mn[:], in0=ps[:], scalar1=b2q[:],
                                        scalar2=0.0, op0=ALU.add, op1=ALU.min)
                e = work.tile([D, 512], F32, tag="fm_e")
                nc.scalar.activation(out=e[:], in_=mn[:], func=ACT.Exp)
                nc.vector.tensor_tensor(out=qt[:, s0:s1], in0=e[:], in1=r[:],
                                        op=ALU.add)

            # ---- cross-core carry via AllGather of local totals ----
            nmods = len(MODS)
            totals = const.tile([D, D * nmods], F32, tag="totals")
            for mi, (m, _) in enumerate(MODS):
                ls = mt[m]["ls"]
                nc.scalar.copy(out=totals[:, mi * D:(mi + 1) * D],
                               in_=ls[:, nch[m] * D:(nch[m] + 1) * D])
            cc_in = dram.tile([D, D * nmods], F32)
            cc_out = dram.tile([N_CORES, D, D * nmods], F32)
            nc.sync.dma_start(out=cc_in[:], in_=totals[:])
            nc.gpsimd.collective_compute(
                "AllGather", ALU.bypass,
                replica_groups=[list(range(N_CORES))],
                ins=[cc_in.opt()], outs=[cc_out.opt()],
            )
            gath = const.tile([D, N_CORES * D * nmods], F32, tag="gath")
            nc.sync.dma_start(out=gath[:],
                              in_=cc_out.rearrange("c p f -> p c f"))
            carry = const.tile([D, D * nmods], F32, tag="carry")
            nc.vector.memset(carry[:], 0.0)
            for cp in range(N_CORES):
                tmp = work.tile([D, D * nmods], F32, tag="carry_tmp")
                g0 = cp * D * nmods
                nc.vector.tensor_scalar(out=tmp[:], in0=gath[:, g0:g0 + D * nmods],
                                        scalar1=prefmask[:, cp:cp + 1], scalar2=None,
                                        op0=ALU.mult)
                nc.vector.tensor_tensor(out=carry[:], in0=carry[:], in1=tmp[:],
                                        op=ALU.add)
            carry_tot = const.tile([D, D], F32, tag="carry_tot")
            nc.vector.tensor_tensor(out=carry_tot[:], in0=carry[:, 0:D],
                                    in1=carry[:, D:2 * D], op=ALU.add)
            nc.vector.tensor_tensor(out=carry_tot[:], in0=carry_tot[:],
                                    in1=carry[:, 2 * D:3 * D], op=ALU.add)

            # ---- output: per ref chunk ----
            for I in range(RC):
                i0 = I * 128
                # selected prefix state: carry + sum_m LS_m[j0_I]
                ssel = work.tile([D, D], F32, tag="ssel")
                first = True
                acc_in = carry_tot
                for m, _ in MODS:
                    j0 = int(bands[m][0][I])
                    nc.vector.tensor_tensor(
                        out=ssel[:], in0=acc_in[:],
                        in1=mt[m]["ls"][:, j0 * D:(j0 + 1) * D], op=ALU.add)
                    acc_in = ssel
                    first = False

                pairs = []
                for m, _ in MODS:
                    j0, j1 = int(bands[m][0][I]), int(bands[m][1][I])
                    for j in range(j0, j1):
                        pairs.append((m, j))

                o_ps = ps_o.tile([128, D], F32, tag="o")
                nc.tensor.matmul(o_ps[:], qt[:, i0:i0 + 128], ssel[:],
                                 start=True, stop=(len(pairs) == 0))
                for pi, (m, j) in enumerate(pairs):
                    v = mt[m]
                    at = ps_at.tile([128, 128], F32, tag="at")
                    nc.tensor.matmul(at[:], v["kt"][:, j * 128:(j + 1) * 128],
                                     qt[:, i0:i0 + 128], start=True, stop=True)
                    msk = work.tile([128, 128], F32, tag="msk")
                    nc.vector.tensor_scalar(out=msk[:], in0=t1b[:, i0:i0 + 128],
                                            scalar1=v["t2n"][:, j:j + 1],
                                            scalar2=None, op0=ALU.is_ge)
                    am = work.tile([128, 128], F32, tag="am")
                    nc.vector.tensor_tensor(out=am[:], in0=at[:], in1=msk[:],
                                            op=ALU.mult)
                    nc.tensor.matmul(o_ps[:], am[:], v["vn"][:, j * D:(j + 1) * D],
                                     start=False, stop=(pi == len(pairs) - 1))
                o_sb = work.tile([128, D], F32, tag="o_sb")
                nc.scalar.copy(out=o_sb[:], in_=o_ps[:])
                nc.sync.dma_start(out=out_t[i0:i0 + 128, :], in_=o_sb[:])

    nc.compile()
    return nc


# ---------------- host-side sharding ----------------

def prepare(inputs):
    """Shard + lay out inputs for the 8 cores. Returns (nch, bands, in_maps)."""
    X = {m: np.ascontiguousarray(np.asarray(inputs["X_" + m], np.float32)[0, 0])
         for m, _ in MODS}
    params = inputs["params"]
    t1 = X["reference"][:, -1]
    bounds = t1[R::R]  # b_1..b_7

    nch = dict(BASE_NCH)
    shards = {}
    for m, d in MODS:
        Xm = X[m]
        t2 = Xm[:, -1]
        s = np.concatenate([[0], np.searchsorted(t2, bounds, side="right"),
                            [len(t2)]]).astype(np.int64)
        max_n = int((s[1:] - s[:-1]).max())
        while nch[m] * 128 < max_n:
            nch[m] += 1
        cap = nch[m] * 128
        per_core = []
        for c in range(N_CORES):
            n_c = int(s[c + 1] - s[c])
            pad = np.zeros((cap, d), np.float32)
            pad[:n_c] = Xm[s[c]:s[c + 1]]
            pad[n_c:, -1] = PAD_T
            per_core.append((pad, n_c))
        shards[m] = per_core

    bands = {}
    for m, _ in MODS:
        ncm = nch[m]
        j0 = np.full(RC, ncm, np.int64)
        j1 = np.zeros(RC, np.int64)
        for c in range(N_CORES):
            tl = shards[m][c][0][:, -1]
            t1c = t1[c * R:(c + 1) * R]
            t1min = t1c[0::128]
            t1max = t1c[127::128]
            fb = np.searchsorted(tl, t1min, side="right")
            lg = np.searchsorted(tl, t1max, side="right")
            j0 = np.minimum(j0, fb // 128)
            j1 = np.maximum(j1, -(-lg // 128))
        j1 = np.minimum(np.maximum(j1, j0), ncm)
        bands[m] = (j0, j1)

    wq1, bq1, wq2, bq2 = _mlp_layers(params["W_Q"])
    shared = {
        "w1q": wq1, "b1q": bq1[:, None], "w2q": wq2, "b2q": bq2[:, None],
        "ident": np.eye(D, dtype=np.float32),
    }
    for m, d in MODS:
        k1, kb1, k2, kb2 = _mlp_layers(params["W_K"][m])
        v1, vb1, v2, vb2 = _mlp_layers(params["W_V"][m])
        shared[f"w1kv_{m}"] = np.concatenate([k1, v1], axis=1)
        shared[f"b1kv_{m}"] = np.concatenate([kb1, vb1])[:, None]
        shared[f"w2k_{m}"] = k2
        shared[f"b2k_{m}"] = kb2[:, None]
        shared[f"w2v_{m}"] = v2
        shared[f"b2v_{m}"] = vb2[:, None]

    in_maps = []
    for c in range(N_CORES):
        im = dict(shared)
        im["xq_t"] = np.ascontiguousarray(X["reference"][c * R:(c + 1) * R].T)
        im["prefmask"] = np.tile((np.arange(N_CORES) < c).astype(np.float32),
                                 (D, 1))
        for m, d in MODS:
            pad, n_c = shards[m][c]
            cap = nch[m] * 128
            im[f"x_{m}_t"] = np.ascontiguousarray(pad.T)
            im[f"t2n_{m}"] = np.ascontiguousarray(
                pad[:, -1].reshape(nch[m], 128).T)
            im[f"rmask_{m}"] = np.ascontiguousarray(
                (np.arange(cap) < n_c).astype(np.float32).reshape(nch[m], 128).T)
        in_maps.append(im)
    return nch, bands, in_maps


_CACHE = {}


def get_program(nch, bands):
    key = (tuple(sorted(nch.items())),
           tuple((m, tuple(map(int, b[0])), tuple(map(int, b[1])))
                 for m, b in sorted(bands.items())))
    if key not in _CACHE:
        _CACHE[key] = build_program(nch, bands)
    return _CACHE[key]


def kernel(**inputs):
    nch, bands, in_maps = prepare(inputs)
    nc = get_program(nch, bands)
    res = run_bass_kernel_spmd(nc, in_maps, core_ids=list(range(N_CORES)))
    out = np.concatenate([res.results[c]["o"] for c in range(N_CORES)], axis=0)
    return out[None]
